# revision 6
# baseline (speedup 1.0000x reference)
"""Trainium2 Bass kernel: BinaryBasicBlock (binary 3x3 conv + train-mode BN + residual).

Math (matches the jax reference):
  a  = sign(x)                          in {-1, 0, +1}
  bw = scale_o * sign(w)                scale_o = mean|w[o]| (host-computed)
  z  = conv2d(a, sign(w), pad=1)        exact small integers -> exact in fp16
  y  = scale_o * z
  BN train mode over (N, H, W) per channel, then residual add:
  out = (z - mean_z) * A + beta + x,  A = gamma*scale_o / sqrt(scale_o^2*var_z + eps)
      = z*A + B + x,                  B = beta - mean_z*A

Distribution: data-parallel, 4 images per core (8 cores), BN batch statistics
all-reduced across cores via a tiny [128,2] AllReduce.

Per-core device plan (all layouts have partition dim = (img_parity*64 + channel)):
  - 2 "pairs" of images; pair p occupies SBUF tiles with img (2p) channels on
    partitions 0..63 and img (2p+1) on partitions 64..127.
  - sign(x) via one ACT op per pair into a zero-bordered padded buffer (fp8).
  - 3x3 conv = 9 shifted matmuls accumulating in PSUM, K=64(in-ch), M=64(out-ch).
    Image A uses PE quadrant (rows 0-63, cols 0-63), image B quadrant
    (rows 64-127, cols 64-127) -> the two run concurrently on the PE array.
  - z kept on-chip in fp16 (exact: |z| <= 576, even integers).
  - bn_stats/bn_aggr for per-lane moments, AllReduce of (mean, mean^2-ish),
    per-channel affine A, B computed on device.
  - pass 2: x += B (ACT), out = z*A + x (DVE scalar_tensor_tensor), DMA out.
HBM traffic per core = read x (12.8MB) + write out (12.8MB) only.
"""

import sys

if "/opt/trn_rl_repo" not in sys.path:
    sys.path.insert(0, "/opt/trn_rl_repo")

from contextlib import ExitStack

import numpy as np
import ml_dtypes

import concourse.bass as bass
import concourse.tile as tile
from concourse import mybir
from concourse.bass_utils import run_bass_kernel_spmd

AF = mybir.ActivationFunctionType
OP = mybir.AluOpType
F32 = mybir.dt.float32
F16 = mybir.dt.float16
F8 = mybir.dt.float8e4

N_CORES = 8
N_LOC = 4            # images per core
C = 64               # channels (in == out)
H = W = 112
HW = H * W           # 12544
WP = W + 2           # padded width 114
HP = H + 2
HWP = HP * WP        # 12996
EPS = 1e-5
CR = 4               # output rows per conv chunk
CHUNK = CR * W       # 448 fp32 -> one PSUM bank
NCH = H // CR        # 28 chunks per image
P2CH = 1568          # pass-2 chunk (free elems per partition)
NP2 = HW // P2CH     # 8


def _split_multi_waits(nc: bass.Bass) -> None:
    """walrus on this toolchain accepts at most ONE sync wait per engine
    instruction ("Too many sync wait commands"), while Tile attaches all
    required waits to the consuming instruction. Move the extra waits onto
    same-engine nops emitted immediately before the instruction (engine
    queues are FIFO, so semantics are preserved)."""
    for bb in list(nc.main_func.blocks):
        targets = []
        for ins in bb.instructions:
            si = ins.sync_info
            if si is not None and si.on_wait and len(si.on_wait) > 1:
                targets.append(ins)
        if not targets:
            continue
        nop_map = {}
        for ins in targets:
            waits = list(ins.sync_info.on_wait)
            updates = list(ins.sync_info.on_update)
            eng = nc.engines[ins.engine]
            nops = []
            for w in waits[:-1]:
                raw = eng.nop().ins
                raw.sync_info = mybir.SyncInfo(on_wait=[w], on_update=[])
                nops.append(raw)
            ins.sync_info = mybir.SyncInfo(on_wait=[waits[-1]], on_update=updates)
            nop_map[id(ins)] = nops
        all_nops = {id(n) for nops in nop_map.values() for n in nops}
        for bb2 in nc.main_func.blocks:
            kept = [i for i in bb2.instructions if id(i) not in all_nops]
            if len(kept) != len(bb2.instructions):
                bb2.instructions = kept
        new_list = []
        for ins in bb.instructions:
            new_list.extend(nop_map.get(id(ins), ()))
            new_list.append(ins)
        bb.instructions = new_list


def build_nc(n_devices: int) -> bass.Bass:
    nc = bass.Bass(num_devices=n_devices)
    x_d = nc.dram_tensor("x", [N_LOC, C, H, W], F32, kind="ExternalInput")
    w_d = nc.dram_tensor("w8", [128, 9, C], F8, kind="ExternalInput")
    gs_d = nc.dram_tensor("gs", [128, 1], F32, kind="ExternalInput")
    s2_d = nc.dram_tensor("s2", [128, 1], F32, kind="ExternalInput")
    bt_d = nc.dram_tensor("bt", [128, 1], F32, kind="ExternalInput")
    out_d = nc.dram_tensor("out", [N_LOC, C, H, W], F32, kind="ExternalOutput")

    x_flat = x_d[:].rearrange("n c h w -> (n c) (h w)")      # [256, 12544]
    out_flat = out_d[:].rearrange("n c h w -> (n c) (h w)")

    with ExitStack() as ctx:
        tc = ctx.enter_context(tile.TileContext(nc))
        persist = ctx.enter_context(tc.tile_pool(name="persist", bufs=1))
        small = ctx.enter_context(tc.tile_pool(name="small", bufs=1))
        psum = ctx.enter_context(tc.tile_pool(name="psum", bufs=3, space="PSUM"))
        dram = ctx.enter_context(tc.tile_pool(name="dram", bufs=1, space="DRAM"))

        xs = [persist.tile([128, HW], F32, tag=f"xs{p}", name=f"xs{p}") for p in range(2)]
        z = [persist.tile([128, HW], F16, tag=f"z{p}", name=f"z{p}") for p in range(2)]
        w_sb = persist.tile([128, 9, C], F8, tag="w8")
        gs_sb = small.tile([128, 1], F32, tag="gs")
        s2_sb = small.tile([128, 1], F32, tag="s2")
        bt_sb = small.tile([128, 1], F32, tag="bt")
        nc.sync.dma_start(w_sb[:], w_d[:])
        nc.sync.dma_start(gs_sb[:], gs_d[:])
        nc.sync.dma_start(s2_sb[:], s2_d[:])
        nc.sync.dma_start(bt_sb[:], bt_d[:])

        stats = small.tile([128, 2 * NCH, 6], F32, tag="stats")

        with tc.tile_pool(name="apad", bufs=1) as apad_pool:
            for p in range(2):
                apad = apad_pool.tile([128, HWP], F8, tag=f"apad{p}", name=f"apad{p}")
                nc.gpsimd.memset(apad[:], 0.0)
                nc.sync.dma_start(xs[p][:], x_flat[p * 128:(p + 1) * 128, :])
                a3 = apad[:].rearrange("q (h w) -> q h w", w=WP)
                x3 = xs[p][:].rearrange("q (h w) -> q h w", w=W)
                nc.scalar.activation(
                    out=a3[:, 1:H + 1, 1:W + 1], in_=x3[:, :, :], func=AF.Sign
                )
                z3 = z[p][:].rearrange("q (n k) -> q n k", k=CHUNK)
                for c in range(NCH):
                    # Separate banks for the two images' accumulation groups:
                    # start=True clears has_written for the whole bank.
                    psa = psum.tile([128, CHUNK], F32, tag="psa", name=f"psa_{p}_{c}")
                    psb = psum.tile([128, CHUNK], F32, tag="psb", name=f"psb_{p}_{c}")
                    for t in range(9):
                        dy, dx = divmod(t, 3)
                        r0 = CR * c + dy
                        nc.tensor.matmul(
                            psa[0:64, :], w_sb[0:64, t, :],
                            a3[0:64, r0:r0 + CR, dx:dx + W],
                            start=(t == 0), stop=(t == 8),
                        )
                        nc.tensor.matmul(
                            psb[64:128, :], w_sb[64:128, t, :],
                            a3[64:128, r0:r0 + CR, dx:dx + W],
                            start=(t == 0), stop=(t == 8),
                        )
                    nc.scalar.copy(out=z3[0:64, c, :], in_=psa[0:64, :])
                    nc.vector.tensor_copy(z3[64:128, c, :], psb[64:128, :])
                for c in range(NCH):
                    nc.vector.bn_stats(out=stats[:, p * NCH + c, :], in_=z3[:, c, :])

        # Per-lane (mean, E[z^2]) over the 2*NCH records, then global reduce.
        mv = small.tile([128, 2], F32, tag="mv")
        nc.vector.bn_aggr(out=mv[:], in_=stats[:])
        m1m2 = small.tile([128, 2], F32, tag="m1m2")
        nc.vector.tensor_copy(m1m2[:, 0:1], mv[:, 0:1])
        nc.vector.tensor_mul(m1m2[:, 1:2], mv[:, 0:1], mv[:, 0:1])
        nc.vector.tensor_add(m1m2[:, 1:2], m1m2[:, 1:2], mv[:, 1:2])

        cc_in = dram.tile([128, 2], F32, tag="ccin")
        cc_out = dram.tile([128, 2], F32, tag="ccout")
        nc.sync.dma_start(cc_in[:], m1m2[:])
        nc.gpsimd.collective_compute(
            "AllReduce",
            OP.add,
            replica_groups=[list(range(n_devices))],
            ins=[cc_in.opt()],
            outs=[cc_out.opt()],
        )
        sums = small.tile([128, 2], F32, tag="sums")
        nc.sync.dma_start(sums[:], cc_out[:])

        # Combine the two partition halves (lane c and c+64 are the same channel).
        up = small.tile([64, 2], F32, tag="up")
        nc.sync.dma_start(up[:], sums[64:128, :])
        tot = small.tile([64, 2], F32, tag="tot")
        nc.vector.tensor_add(tot[:], sums[0:64, :], up[:])
        nc.vector.tensor_scalar_mul(tot[:], tot[:], 1.0 / (2.0 * n_devices))
        e1 = tot[:, 0:1]
        e2 = tot[:, 1:2]
        varg = small.tile([64, 1], F32, tag="varg")
        nc.vector.tensor_mul(varg[:], e1, e1)
        nc.vector.tensor_tensor(out=varg[:], in0=e2, in1=varg[:], op=OP.subtract)
        nc.vector.tensor_mul(varg[:], varg[:], s2_sb[0:64, :])
        epst = small.tile([64, 1], F32, tag="epst")
        nc.vector.memset(epst[:], EPS)
        nc.scalar.activation(out=varg[:], in_=varg[:], func=AF.Sqrt,
                             bias=epst[:], scale=1.0)
        nc.vector.reciprocal(varg[:], varg[:])
        AB = small.tile([128, 2], F32, tag="AB")
        nc.vector.tensor_mul(AB[0:64, 0:1], gs_sb[0:64, :], varg[:])
        tmpb = small.tile([64, 1], F32, tag="tmpb")
        nc.vector.tensor_mul(tmpb[:], e1, AB[0:64, 0:1])
        nc.vector.tensor_tensor(out=AB[0:64, 1:2], in0=bt_sb[0:64, :],
                                in1=tmpb[:], op=OP.subtract)
        nc.sync.dma_start(AB[64:128, :], AB[0:64, :])

        # Pass 2: out = z*A + (x + B), chunked; in-place bias add on xs.
        with tc.tile_pool(name="yh", bufs=2) as yh_pool:
            A_ap = AB[:, 0:1]
            B_ap = AB[:, 1:2]
            for p in range(2):
                for j in range(NP2):
                    sl = slice(j * P2CH, (j + 1) * P2CH)
                    nc.scalar.activation(out=xs[p][:, sl], in_=xs[p][:, sl],
                                         func=AF.Identity, bias=B_ap, scale=1.0)
                    yh = yh_pool.tile([128, P2CH], F32, tag="yh", name=f"yh_{p}_{j}")
                    nc.vector.scalar_tensor_tensor(
                        out=yh[:], in0=z[p][:, sl], scalar=A_ap,
                        in1=xs[p][:, sl], op0=OP.mult, op1=OP.add,
                    )
                    nc.sync.dma_start(out_flat[p * 128:(p + 1) * 128, sl], yh[:])
    _split_multi_waits(nc)
    return nc


def prep_host_inputs(x, weights, gamma, beta):
    x = np.ascontiguousarray(np.asarray(x, dtype=np.float32))
    w = np.asarray(weights, dtype=np.float32).reshape(C, C, 3, 3)
    gamma = np.asarray(gamma, dtype=np.float32).reshape(C)
    beta = np.asarray(beta, dtype=np.float32).reshape(C)
    scale = np.mean(np.abs(w), axis=(1, 2, 3), dtype=np.float32)
    sw = np.sign(w).astype(np.float32)                      # [O, I, ky, kx]
    swT = np.transpose(sw, (1, 2, 3, 0)).reshape(C, 9, C)   # [i, t, o]
    w8 = np.ascontiguousarray(
        np.concatenate([swT, swT], axis=0)
    ).astype(ml_dtypes.float8_e4m3)
    gs = np.ascontiguousarray(np.tile((gamma * scale)[:, None], (2, 1)))
    s2 = np.ascontiguousarray(np.tile((scale * scale)[:, None], (2, 1)))
    bt = np.ascontiguousarray(np.tile(beta[:, None], (2, 1)))
    return x, w8, gs.astype(np.float32), s2.astype(np.float32), bt.astype(np.float32)


def make_in_maps(x, w8, gs, s2, bt, n_cores):
    return [
        {
            "x": np.ascontiguousarray(x[i * N_LOC:(i + 1) * N_LOC]),
            "w8": w8,
            "gs": gs,
            "s2": s2,
            "bt": bt,
        }
        for i in range(n_cores)
    ]


def kernel(x, weights, gamma, beta):
    x, w8, gs, s2, bt = prep_host_inputs(x, weights, gamma, beta)
    nc = build_nc(N_CORES)
    in_maps = make_in_maps(x, w8, gs, s2, bt, N_CORES)
    res = run_bass_kernel_spmd(nc, in_maps, list(range(N_CORES)))
    out = np.concatenate([res.results[i]["out"] for i in range(N_CORES)], axis=0)
    return out.astype(np.float32)
